# revision 42
# baseline (speedup 1.0000x reference)
"""GCN layer (Linear -> weighted-adjacency SpMM -> BatchNorm(eval) -> exact GELU)
as a Bass/Tile kernel on 8 Trainium2 NeuronCores.

Sharding: both source and destination nodes are sharded across the 8 cores
(12500 rows each).  Phase 1 computes the LOCAL shard of
`support = x @ W' + b'` only (x arrives pre-sharded + transposed + bf16 per
core, 13MB instead of a replicated 205MB f32 copy), written as 2 row-halves.
Each half is AllGathered across cores (two 26MB collectives ride the
high-bandwidth end of the collective size curve).  The collectives and the
16->128 index-replication DMAs are issued on the otherwise-idle ACTIVATION
queue, keeping the in-order gpsimd queue free for gathers -- on-engine
collective time was the critical path otherwise.  The gathered half
[8 x 6400, 256] splits into two 25600-row sections (cores 0-3 / cores 4-7),
keeping sources int16-addressable.

Phase 2 per destination tile (128 rows): source rows of the section are
fetched with one `dma_gather` (int16 section-local indices, runtime
valid-count register, negative-index tail padding), and segment-summed on the
tensor engine with per-128-edge-chunk one-hot selector matmuls accumulating in
PSUM; per-section partials accumulate into an SBUF-resident f32 accumulator
seeded with the folded BN shift.  The selector is built in a transposed layout
sel[p, dest, chunk] so every DVE operand has a stride-1 2-byte last dim --
this enables the DVE 2x perf mode and halves selector-build time; the matmul
reads it back as strided stationary slices sel[:, :, u].  All feature data is
bf16 (halves PCIe streaming and the random-gather HBM traffic that is this
memory-regime problem's roofline); accumulation stays f32 in PSUM/SBUF.  BN is
folded on the host (W' = W*s, shift = beta - mean*s, s = gamma/sqrt(var+eps));
the epilogue per tile is one add + one exact-GELU activation.

Host-side prep inside kernel(): shard + transpose + bf16-cast x; per core
group edges by (source-section, destination-tile); pack per-group edge row/val
into fixed 128-edge chunk layout and indices into the 16-partition-wrapped
int16 layout dma_gather expects.  One SPMD program serves all 8 cores;
per-group edge counts ride in as data.
"""

import sys

sys.path.insert(0, "/opt/trn_rl_repo")

import numpy as np

import concourse.tile as tile
from concourse import bacc, mybir
from concourse.bass import BassGpSimd
from concourse.bass_utils import run_bass_kernel_spmd

F32 = mybir.dt.float32
BF16 = mybir.dt.bfloat16
I32 = mybir.dt.int32
I16 = mybir.dt.int16
AF = mybir.ActivationFunctionType
ALU = mybir.AluOpType
NPBF16 = mybir.dt.np(mybir.dt.bfloat16)

N_CORES = 8
SHARD = 12500   # valid rows per core
SH = 12800      # padded rows per core (100 tiles)
HROWS = 6400    # rows per half (SH/2), 50 tiles
TPS = 14        # dest tiles per index-slab load (must divide nt)
GRP = 1         # dest tiles per merged gather (must divide TPS); GRP>1 pads
                # interior slots with valid index 0, which measurably COSTS
                # real HBM traffic -- gather time is byte-bound, keep GRP=1
NQ = 4          # SWDGE queues to spread gathers over (ucode max 4): a single
                # queue drives only a subset of DMA engines and caps random
                # 512B-row gathers at ~50GB/s/core
XCOLS = 640     # node columns per phase-1 supertile (divides HROWS)
NGBUF = 8       # round-robin gather buffers (each holds GRP tiles' rows)


def _build_program(*, in_dim, out_dim, nt, c_sub, tps, xcols,
                   skip_cc=False, skip_p2=False):
    assert in_dim % 128 == 0 and HROWS % xcols == 0
    assert nt % tps == 0 and tps % GRP == 0
    kb = in_dim // 128
    sec_rows = HROWS * 4                # 25600 (4 cores/section), int16-safe
    nsup_h = HROWS // xcols             # supertiles per half
    jt = xcols // 128
    gsl = c_sub * 128                   # idx slots per dest tile
    nidx = GRP * gsl                    # idxs per merged gather
    idxcols = nidx // 16
    ng = 4 * nt
    npair = ng // GRP                   # merged gathers

    nc = bacc.Bacc("TRN2", target_bir_lowering=False, debug=False,
                   num_devices=N_CORES, num_swdge_queues=NQ)

    xT = nc.dram_tensor("xT", [in_dim, SH], BF16, kind="ExternalInput").ap()
    Wp = nc.dram_tensor("Wp", [in_dim, out_dim], BF16, kind="ExternalInput").ap()
    bp = nc.dram_tensor("bp", [128, out_dim], F32, kind="ExternalInput").ap()
    shiftb = nc.dram_tensor("shiftb", [128, out_dim], F32, kind="ExternalInput").ap()
    iota2_in = nc.dram_tensor("iota2", [128, 128, 2 * c_sub], BF16,
                              kind="ExternalInput").ap()
    idx16 = nc.dram_tensor("idx16", [16, npair * idxcols], I16,
                           kind="ExternalInput").ap()
    rowp = nc.dram_tensor("rowp", [128, ng * c_sub], BF16,
                          kind="ExternalInput").ap()
    valp = nc.dram_tensor("valp", [128, ng * c_sub], BF16,
                          kind="ExternalInput").ap()
    cnts = nc.dram_tensor("cnts", [1, npair], I32, kind="ExternalInput").ap()
    out = nc.dram_tensor("out", [nt * 128, out_dim], BF16,
                         kind="ExternalOutput").ap()
    sup_local = nc.dram_tensor("sup_local", [SH, out_dim], BF16).ap()
    # one replicated-index tensor per section: no false cross-section DRAM
    # dependencies between the replication DMAs and earlier passes' reads
    idxps = [nc.dram_tensor(f"idxp{s}", [128, (nt // GRP) * idxcols], I16).ap()
             for s in range(4)]
    allg = [nc.dram_tensor(f"allg{h}", [N_CORES * HROWS, out_dim], BF16,
                           addr_space="Shared").ap()
            for h in range(2)]

    # section s (0..3) reads rows of allg[s>>1][(s&1)*sec_rows : +sec_rows]
    def sec_ap(s):
        return allg[s >> 1][(s & 1) * sec_rows:(s & 1) * sec_rows + sec_rows, :]

    with tile.TileContext(nc) as tc, tc.tile_pool(name="consts", bufs=1) as consts:
        w_sb = consts.tile([128, kb, out_dim], BF16)
        bp_sb = consts.tile([128, out_dim], F32)
        shift_sb = consts.tile([128, out_dim], F32)
        iota2_sb = consts.tile([128, 128, 2 * c_sub], BF16)
        cnt_sb = consts.tile([128, npair], I32)
        acc = consts.tile([128, nt, out_dim], F32)
        gts = consts.tile([128, NGBUF, GRP * c_sub, out_dim], BF16)
        for i in range(NGBUF):
            nc.vector.memset(gts[:, i], 0.0)
        for k in range(kb):
            nc.sync.dma_start(w_sb[:, k, :], Wp[k * 128:(k + 1) * 128, :])
        nc.sync.dma_start(bp_sb[:], bp[:])
        nc.sync.dma_start(shift_sb[:], shiftb[:])
        nc.sync.dma_start(iota2_sb[:], iota2_in[:])
        nc.sync.dma_start(cnt_sb[0:1, :], cnts[:])

        # Phase-2 pools opened first: disjoint SBUF from phase-1 pools, so
        # phase-2 allocations carry no WAR deps on phase-1 releases.
        with (
            tc.tile_pool(name="slabs", bufs=2) as slabs,
            tc.tile_pool(name="sel", bufs=2) as selpool,
            tc.tile_pool(name="p2psum", bufs=6, space="PSUM") as p2psum,
        ):
            with (
                tc.tile_pool(name="xt", bufs=2) as xpool,
                tc.tile_pool(name="p1psum", bufs=2, space="PSUM") as p1psum,
                tc.tile_pool(name="p1out", bufs=4) as p1out,
            ):
                def p1_half(h):
                    for st in range(nsup_h):
                        gcol = (h * nsup_h + st) * xcols
                        xt = xpool.tile([128, kb, xcols], BF16)
                        for k in range(kb):
                            nc.sync.dma_start(
                                xt[:, k, :],
                                xT[k * 128:(k + 1) * 128, gcol:gcol + xcols])
                        for j in range(jt):
                            ps = p1psum.tile([128, out_dim], F32)
                            for k in range(kb):
                                nc.tensor.matmul(
                                    ps[:], lhsT=xt[:, k, j * 128:(j + 1) * 128],
                                    rhs=w_sb[:, k, :],
                                    start=(k == 0), stop=(k == kb - 1))
                            so = p1out.tile([128, out_dim], BF16)
                            nc.vector.tensor_tensor(so[:], ps[:], bp_sb[:],
                                                    op=ALU.add)
                            r0 = gcol + j * 128
                            nc.sync.dma_start(sup_local[r0:r0 + 128, :], so[:])

                def allgather(h):
                    if skip_cc:
                        return
                    # collectives are only legal on the gpsimd engine (BIR
                    # verifier); emission order keeps them off the gather
                    # critical path: AG0 | pass0 | pass1 | AG1 | pass2 | pass3
                    nc.gpsimd.collective_compute(
                        "AllGather",
                        ALU.bypass,
                        replica_groups=[list(range(N_CORES))],
                        ins=[sup_local[h * HROWS:(h + 1) * HROWS, :]],
                        outs=[allg[h][:]],
                    )

                def idx_replicate(s):
                    # 16 -> 128 partition replication for section s's indices.
                    # On the SP queue: executes right after phase-1's loads,
                    # well before any pass needs it -- putting it on the
                    # activation queue would run it behind the collectives
                    # while its DMA-ring slots throttle later slab loads.
                    c0 = s * (nt // GRP) * idxcols
                    c1 = (s + 1) * (nt // GRP) * idxcols
                    for g in range(8):
                        nc.sync.dma_start(idxps[s][16 * g:16 * (g + 1), :],
                                          idx16[:, c0:c1])

                nreg = nc.gpsimd.alloc_register("gcnt")
                gbuf_i = 0

                def p2_pass(s):
                    nonlocal gbuf_i
                    for sl in range(nt // tps):
                        idx_sb = slabs.tile([128, (tps // GRP) * idxcols], I16,
                                            tag="idx")
                        row_sb = slabs.tile([128, tps * c_sub], BF16, tag="row")
                        val_sb = slabs.tile([128, tps * c_sub], BF16, tag="val")
                        gbase = s * nt + sl * tps
                        pbase = (s * nt + sl * tps) // GRP
                        lpbase = (sl * tps) // GRP
                        nc.sync.dma_start(
                            idx_sb[:],
                            idxps[s][:, lpbase * idxcols:
                                     (lpbase + tps // GRP) * idxcols])
                        nc.sync.dma_start(
                            row_sb[:], rowp[:, gbase * c_sub:(gbase + tps) * c_sub])
                        nc.sync.dma_start(
                            val_sb[:], valp[:, gbase * c_sub:(gbase + tps) * c_sub])
                        # process dest tiles in PAIRS: two gathers issue back
                        # to back (round-robin queues), then ONE is_equal +
                        # ONE mult build both tiles' selectors [128,128,2*c]
                        # -- halves DVE dispatch overhead, keeps 2x mode
                        for pp in range(tps // 2):
                            tl0 = pp * 2
                            gtp = []
                            for a in range(2):
                                pair = pbase + tl0 + a
                                gt = gts[:, gbuf_i % NGBUF]
                                gbuf_i += 1
                                nc.gpsimd.reg_load(nreg,
                                                   cnt_sb[0:1, pair:pair + 1])
                                nc.gpsimd.dma_gather(
                                    out_ap=gt[:],
                                    in_ap=sec_ap(s),
                                    idxs_ap=idx_sb[:, (tl0 + a) * idxcols:
                                                   (tl0 + a + 1) * idxcols],
                                    num_idxs=nidx,
                                    num_idxs_reg=nreg,
                                    elem_size=out_dim,
                                    single_packet=False,
                                    queue_num=gbuf_i % NQ,
                                )
                                gtp.append(gt)
                            # sel[p, d, a*c_sub+c] = (row[p,a,c] == d) * val
                            sel = selpool.tile([128, 128, 2 * c_sub], BF16,
                                               tag="sel")
                            row3 = row_sb[:, (tl0 * c_sub):(tl0 + 2) * c_sub] \
                                .unsqueeze(1).to_broadcast([128, 128, 2 * c_sub])
                            val3 = val_sb[:, (tl0 * c_sub):(tl0 + 2) * c_sub] \
                                .unsqueeze(1).to_broadcast([128, 128, 2 * c_sub])
                            nc.vector.tensor_tensor(sel[:], row3, iota2_sb[:],
                                                    op=ALU.is_equal)
                            nc.vector.tensor_tensor(sel[:], sel[:], val3,
                                                    op=ALU.mult)
                            for a in range(2):
                                t = sl * tps + tl0 + a
                                ps = p2psum.tile([128, out_dim], F32)
                                for u in range(c_sub):
                                    nc.tensor.matmul(
                                        ps[:], lhsT=sel[:, :, a * c_sub + u],
                                        rhs=gtp[a][:, u, :],
                                        start=(u == 0), stop=(u == c_sub - 1))
                                if s == 0:
                                    # seed with the folded BN shift: saves an
                                    # add in the epilogue
                                    nc.vector.tensor_tensor(
                                        acc[:, t, :], ps[:], shift_sb[:],
                                        op=ALU.add)
                                elif s < 3:
                                    nc.vector.tensor_tensor(
                                        acc[:, t, :], acc[:, t, :], ps[:],
                                        op=ALU.add)
                                else:
                                    ob = selpool.tile([128, out_dim], F32,
                                                      tag="ob")
                                    nc.vector.tensor_tensor(ob[:], acc[:, t, :],
                                                            ps[:], op=ALU.add)
                                    ob3 = selpool.tile([128, out_dim], BF16,
                                                      tag="ob3")
                                    nc.scalar.activation(ob3[:], ob[:], AF.Gelu)
                                    nc.sync.dma_start(
                                        out[t * 128:(t + 1) * 128, :], ob3[:])

                # Pool queue: AG0 | pass0+1 gathers | AG1 | pass2+3 gathers.
                # Passes 0/1 read half-0 sections, passes 2/3 half-1, so only
                # passes that genuinely need AG1 queue behind it.
                p1_half(0)
                allgather(0)
                p1_half(1)
                idx_replicate(0)
                idx_replicate(1)
                idx_replicate(2)
                idx_replicate(3)
                if not skip_p2:
                    p2_pass(0)
                    p2_pass(1)
                # scheduling hint: don't let the greedy scheduler slot AG1
                # onto the Pool queue before pass-0/1's gathers (deps alone
                # would allow it at t~350us, serializing everything behind it)
                with tc.tile_wait_until(0.75):
                    allgather(1)
                if not skip_p2:
                    p2_pass(2)
                    p2_pass(3)

    nc.compile()
    return nc


def _preprocess(x, edge_row, edge_col, edge_val, W, b, gamma, beta,
                running_mean, running_var, bn_eps=1e-5):
    n, in_dim = x.shape
    out_dim = W.shape[1]
    assert n == N_CORES * SHARD
    nt = (SHARD + 127) // 128
    nt = ((nt + TPS - 1) // TPS) * TPS
    ng = 4 * nt

    inv_std = 1.0 / np.sqrt(running_var.astype(np.float64) + bn_eps)
    scale = (inv_std * gamma.astype(np.float64)).astype(np.float32)
    shift = (beta.astype(np.float64) - running_mean.astype(np.float64) * inv_std
             * gamma.astype(np.float64)).astype(np.float32)

    xb = x.astype(NPBF16)
    Wp = (W * scale[None, :]).astype(NPBF16)
    bp = np.ascontiguousarray(
        np.broadcast_to((b * scale).astype(np.float32), (128, out_dim)))
    shiftb = np.ascontiguousarray(np.broadcast_to(shift, (128, out_dim)))

    per_core = []
    c_sub = 1
    for m in range(N_CORES):
        lo, hi = m * SHARD, (m + 1) * SHARD
        mask = (edge_row >= lo) & (edge_row < hi)
        er = (edge_row[mask] - lo).astype(np.int64)
        ec = edge_col[mask].astype(np.int64)
        ev = edge_val[mask].astype(np.float32)
        src_core = ec // SHARD
        src_r = ec % SHARD
        h = src_r // HROWS
        sec = h * 2 + (src_core >> 2)
        loc = (src_core & 3) * HROWS + (src_r - h * HROWS)
        gid = sec * nt + (er >> 7)
        order = np.argsort(gid, kind="stable")
        er, ev, loc, gid = er[order], ev[order], loc[order], gid[order]
        counts = np.bincount(gid, minlength=ng)
        per_core.append((er, ev, loc, gid, counts))
        c_sub = max(c_sub, int(((counts + 127) // 128).max()))
    gsl = c_sub * 128
    nidx = GRP * gsl
    idxcols = nidx // 16
    npair = ng // GRP

    # iota2[p, d, c] = d  (transposed-selector compare plane, 2-tile wide)
    iota2 = np.ascontiguousarray(np.broadcast_to(
        np.repeat(np.arange(128, dtype=np.float32),
                  2 * c_sub).reshape(1, 128, 2 * c_sub),
        (128, 128, 2 * c_sub))).astype(NPBF16)

    in_maps = []
    for m in range(N_CORES):
        er, ev, loc, gid, counts = per_core[m]
        starts = np.zeros(ng, np.int64)
        np.cumsum(counts[:-1], out=starts[1:])
        rank = np.arange(len(er)) - starts[gid]
        rowp = np.zeros((128, ng * c_sub), NPBF16)
        valp = np.zeros((128, ng * c_sub), NPBF16)
        rowp[rank & 127, gid * c_sub + (rank >> 7)] = (er & 127).astype(NPBF16)
        valp[rank & 127, gid * c_sub + (rank >> 7)] = ev.astype(NPBF16)
        # merged-gather index stream: GRP tile-groups per gather; non-final
        # groups pad with VALID index 0 (rowp/valp stay 0 there, so the
        # selector zeroes those contributions); the final group pads with -1
        # which the count register trims.
        idx16 = np.full((16, npair * idxcols), -1, np.int16)
        idx16.reshape(16, npair, GRP, gsl // 16)[:, :, :GRP - 1, :] = 0
        slot = (gid % GRP) * gsl + rank
        idx16[slot & 15, (gid // GRP) * idxcols + (slot >> 4)] = \
            loc.astype(np.int16)
        cnts_arr = ((GRP - 1) * gsl +
                    counts.reshape(npair, GRP)[:, GRP - 1]).astype(np.int32)
        empty = np.nonzero(cnts_arr == 0)[0]
        if len(empty):
            idx16[0, empty * idxcols] = 0  # one dummy valid index, val stays 0
            cnts_arr[empty] = 1

        xTm = np.zeros((in_dim, SH), NPBF16)
        xTm[:, :SHARD] = xb[m * SHARD:(m + 1) * SHARD].T
        in_maps.append({
            "xT": np.ascontiguousarray(xTm),
            "Wp": Wp, "bp": bp, "shiftb": shiftb, "iota2": iota2,
            "idx16": np.ascontiguousarray(idx16),
            "rowp": np.ascontiguousarray(rowp),
            "valp": np.ascontiguousarray(valp),
            "cnts": cnts_arr.reshape(1, npair),
        })

    params = dict(in_dim=in_dim, out_dim=out_dim, nt=nt, c_sub=c_sub,
                  tps=TPS, xcols=XCOLS)
    return in_maps, params, SHARD


def kernel(x, edge_row, edge_col, edge_val, W, b, gamma, beta,
           running_mean, running_var):
    x = np.asarray(x)
    edge_row = np.asarray(edge_row)
    edge_col = np.asarray(edge_col)
    edge_val = np.asarray(edge_val)
    W = np.asarray(W)
    b = np.asarray(b)
    gamma = np.asarray(gamma)
    beta = np.asarray(beta)
    running_mean = np.asarray(running_mean)
    running_var = np.asarray(running_var)

    in_maps, params, shard = _preprocess(
        x, edge_row, edge_col, edge_val, W, b, gamma, beta,
        running_mean, running_var)
    nc = _build_program(**params)
    res = run_bass_kernel_spmd(nc, in_maps, core_ids=list(range(N_CORES)))
    outs = [res.results[m]["out"][:shard].astype(np.float32)
            for m in range(N_CORES)]
    return np.concatenate(outs, axis=0)


# revision 45
# speedup vs baseline: 1.0889x; 1.0889x over previous
"""GCN layer (Linear -> weighted-adjacency SpMM -> BatchNorm(eval) -> exact GELU)
as a Bass/Tile kernel on 8 Trainium2 NeuronCores.

Sharding: both source and destination nodes are sharded across the 8 cores
(12500 rows each).  Phase 1 computes the LOCAL shard of
`support = x @ W' + b'` only (x arrives pre-sharded + transposed + bf16 per
core, 13MB instead of a replicated 205MB f32 copy), written as 2 row-halves.
Each half is AllGathered across cores (two 26MB collectives ride the
high-bandwidth end of the collective size curve).  The collectives and the
16->128 index-replication DMAs are issued on the otherwise-idle ACTIVATION
queue, keeping the in-order gpsimd queue free for gathers -- on-engine
collective time was the critical path otherwise.  The gathered half
[8 x 6400, 256] splits into two 25600-row sections (cores 0-3 / cores 4-7),
keeping sources int16-addressable.

Phase 2 per destination tile (128 rows): source rows of the section are
fetched with one `dma_gather` (int16 section-local indices, runtime
valid-count register, negative-index tail padding), and segment-summed on the
tensor engine with per-128-edge-chunk one-hot selector matmuls accumulating in
PSUM; per-section partials accumulate into an SBUF-resident f32 accumulator
seeded with the folded BN shift.  The selector is built in a transposed layout
sel[p, dest, chunk] so every DVE operand has a stride-1 2-byte last dim --
this enables the DVE 2x perf mode and halves selector-build time; the matmul
reads it back as strided stationary slices sel[:, :, u].  All feature data is
bf16 (halves PCIe streaming and the random-gather HBM traffic that is this
memory-regime problem's roofline); accumulation stays f32 in PSUM/SBUF.  BN is
folded on the host (W' = W*s, shift = beta - mean*s, s = gamma/sqrt(var+eps));
the epilogue per tile is one add + one exact-GELU activation.

Host-side prep inside kernel(): shard + transpose + bf16-cast x; per core
group edges by (source-section, destination-tile); pack per-group edge row/val
into fixed 128-edge chunk layout and indices into the 16-partition-wrapped
int16 layout dma_gather expects.  One SPMD program serves all 8 cores;
per-group edge counts ride in as data.
"""

import sys

sys.path.insert(0, "/opt/trn_rl_repo")

import numpy as np

import concourse.tile as tile
from concourse import bacc, mybir
from concourse.bass import BassGpSimd
from concourse.bass_utils import run_bass_kernel_spmd

F32 = mybir.dt.float32
BF16 = mybir.dt.bfloat16
I32 = mybir.dt.int32
I16 = mybir.dt.int16
AF = mybir.ActivationFunctionType
ALU = mybir.AluOpType
NPBF16 = mybir.dt.np(mybir.dt.bfloat16)

N_CORES = 8
SHARD = 12500   # valid rows per core
SH = 12800      # padded rows per core (100 tiles)
HROWS = 6400    # rows per half (SH/2), 50 tiles
TPS = 14        # dest tiles per index-slab load (must divide nt)
GRP = 1         # dest tiles per merged gather (must divide TPS); GRP>1 pads
                # interior slots with valid index 0, which measurably COSTS
                # real HBM traffic -- gather time is byte-bound, keep GRP=1
NQ = 4          # SWDGE queues to spread gathers over (ucode max 4): a single
                # queue drives only a subset of DMA engines and caps random
                # 512B-row gathers at ~50GB/s/core
XCOLS = 640     # node columns per phase-1 supertile (divides HROWS)
NGBUF = 10      # round-robin gather buffers (each holds GRP tiles' rows):
                # deep ring keeps all 4 SWDGE queues fed ahead of consumption


def _build_program(*, in_dim, out_dim, nt, c_sub, tps, xcols,
                   skip_cc=False, skip_p2=False):
    assert in_dim % 128 == 0 and HROWS % xcols == 0
    assert nt % tps == 0 and tps % GRP == 0
    kb = in_dim // 128
    sec_rows = HROWS * 4                # 25600 (4 cores/section), int16-safe
    nsup_h = HROWS // xcols             # supertiles per half
    jt = xcols // 128
    gsl = c_sub * 128                   # idx slots per dest tile
    nidx = GRP * gsl                    # idxs per merged gather
    idxcols = nidx // 16
    ng = 4 * nt
    npair = ng // GRP                   # merged gathers

    nc = bacc.Bacc("TRN2", target_bir_lowering=False, debug=False,
                   num_devices=N_CORES, num_swdge_queues=NQ)

    xT = nc.dram_tensor("xT", [in_dim, SH], BF16, kind="ExternalInput").ap()
    Wp = nc.dram_tensor("Wp", [in_dim, out_dim], BF16, kind="ExternalInput").ap()
    bp = nc.dram_tensor("bp", [128, out_dim], F32, kind="ExternalInput").ap()
    shiftb = nc.dram_tensor("shiftb", [128, out_dim], F32, kind="ExternalInput").ap()
    iota2_in = nc.dram_tensor("iota2", [128, 128, c_sub], BF16,
                              kind="ExternalInput").ap()
    idx16 = nc.dram_tensor("idx16", [16, npair * idxcols], I16,
                           kind="ExternalInput").ap()
    rowp = nc.dram_tensor("rowp", [128, ng * c_sub], BF16,
                          kind="ExternalInput").ap()
    valp = nc.dram_tensor("valp", [128, ng * c_sub], BF16,
                          kind="ExternalInput").ap()
    cnts = nc.dram_tensor("cnts", [1, npair], I32, kind="ExternalInput").ap()
    out = nc.dram_tensor("out", [nt * 128, out_dim], BF16,
                         kind="ExternalOutput").ap()
    sup_local = nc.dram_tensor("sup_local", [SH, out_dim], BF16).ap()
    # one replicated-index tensor per section: no false cross-section DRAM
    # dependencies between the replication DMAs and earlier passes' reads
    idxps = [nc.dram_tensor(f"idxp{s}", [128, (nt // GRP) * idxcols], I16).ap()
             for s in range(4)]
    allg = [nc.dram_tensor(f"allg{h}", [N_CORES * HROWS, out_dim], BF16,
                           addr_space="Shared").ap()
            for h in range(2)]

    # section s (0..3) reads rows of allg[s>>1][(s&1)*sec_rows : +sec_rows]
    def sec_ap(s):
        return allg[s >> 1][(s & 1) * sec_rows:(s & 1) * sec_rows + sec_rows, :]

    with tile.TileContext(nc) as tc, tc.tile_pool(name="consts", bufs=1) as consts:
        w_sb = consts.tile([128, kb, out_dim], BF16)
        bp_sb = consts.tile([128, out_dim], F32)
        shift_sb = consts.tile([128, out_dim], F32)
        iota2_sb = consts.tile([128, 128, c_sub], BF16)
        cnt_sb = consts.tile([128, npair], I32)
        acc = consts.tile([128, nt, out_dim], F32)
        gts = consts.tile([128, NGBUF, GRP * c_sub, out_dim], BF16)
        for i in range(NGBUF):
            nc.vector.memset(gts[:, i], 0.0)
        for k in range(kb):
            nc.sync.dma_start(w_sb[:, k, :], Wp[k * 128:(k + 1) * 128, :])
        nc.sync.dma_start(bp_sb[:], bp[:])
        nc.sync.dma_start(shift_sb[:], shiftb[:])
        nc.sync.dma_start(iota2_sb[:], iota2_in[:])
        nc.sync.dma_start(cnt_sb[0:1, :], cnts[:])

        # Phase-2 pools opened first: disjoint SBUF from phase-1 pools, so
        # phase-2 allocations carry no WAR deps on phase-1 releases.
        with (
            tc.tile_pool(name="slabs", bufs=3) as slabs,
            tc.tile_pool(name="sel", bufs=2) as selpool,
            tc.tile_pool(name="p2psum", bufs=6, space="PSUM") as p2psum,
        ):
            with (
                tc.tile_pool(name="xt", bufs=2) as xpool,
                tc.tile_pool(name="p1psum", bufs=2, space="PSUM") as p1psum,
                tc.tile_pool(name="p1out", bufs=4) as p1out,
            ):
                def p1_half(h):
                    for st in range(nsup_h):
                        gcol = (h * nsup_h + st) * xcols
                        xt = xpool.tile([128, kb, xcols], BF16)
                        for k in range(kb):
                            nc.sync.dma_start(
                                xt[:, k, :],
                                xT[k * 128:(k + 1) * 128, gcol:gcol + xcols])
                        for j in range(jt):
                            ps = p1psum.tile([128, out_dim], F32)
                            for k in range(kb):
                                nc.tensor.matmul(
                                    ps[:], lhsT=xt[:, k, j * 128:(j + 1) * 128],
                                    rhs=w_sb[:, k, :],
                                    start=(k == 0), stop=(k == kb - 1))
                            so = p1out.tile([128, out_dim], BF16)
                            nc.vector.tensor_tensor(so[:], ps[:], bp_sb[:],
                                                    op=ALU.add)
                            r0 = gcol + j * 128
                            nc.sync.dma_start(sup_local[r0:r0 + 128, :], so[:])

                def allgather(h):
                    if skip_cc:
                        return
                    # collectives are only legal on the gpsimd engine (BIR
                    # verifier); emission order keeps them off the gather
                    # critical path: AG0 | pass0 | pass1 | AG1 | pass2 | pass3
                    nc.gpsimd.collective_compute(
                        "AllGather",
                        ALU.bypass,
                        replica_groups=[list(range(N_CORES))],
                        ins=[sup_local[h * HROWS:(h + 1) * HROWS, :]],
                        outs=[allg[h][:]],
                    )

                def idx_replicate(s):
                    # 16 -> 128 partition replication for section s's indices.
                    # On the SP queue: executes right after phase-1's loads,
                    # well before any pass needs it -- putting it on the
                    # activation queue would run it behind the collectives
                    # while its DMA-ring slots throttle later slab loads.
                    c0 = s * (nt // GRP) * idxcols
                    c1 = (s + 1) * (nt // GRP) * idxcols
                    for g in range(8):
                        nc.sync.dma_start(idxps[s][16 * g:16 * (g + 1), :],
                                          idx16[:, c0:c1])

                nreg = nc.gpsimd.alloc_register("gcnt")
                gbuf_i = 0

                def p2_pass(s):
                    nonlocal gbuf_i
                    for sl in range(nt // tps):
                        idx_sb = slabs.tile([128, (tps // GRP) * idxcols], I16,
                                            tag="idx")
                        row_sb = slabs.tile([128, tps * c_sub], BF16, tag="row")
                        val_sb = slabs.tile([128, tps * c_sub], BF16, tag="val")
                        gbase = s * nt + sl * tps
                        pbase = (s * nt + sl * tps) // GRP
                        lpbase = (sl * tps) // GRP
                        nc.sync.dma_start(
                            idx_sb[:],
                            idxps[s][:, lpbase * idxcols:
                                     (lpbase + tps // GRP) * idxcols])
                        nc.sync.dma_start(
                            row_sb[:], rowp[:, gbase * c_sub:(gbase + tps) * c_sub])
                        nc.sync.dma_start(
                            val_sb[:], valp[:, gbase * c_sub:(gbase + tps) * c_sub])
                        for pp in range(tps // GRP):
                            pair = pbase + pp
                            gt = gts[:, gbuf_i % NGBUF]
                            gbuf_i += 1
                            nc.gpsimd.reg_load(nreg, cnt_sb[0:1, pair:pair + 1])
                            nc.gpsimd.dma_gather(
                                out_ap=gt[:],
                                in_ap=sec_ap(s),
                                idxs_ap=idx_sb[:, pp * idxcols:(pp + 1) * idxcols],
                                num_idxs=nidx,
                                num_idxs_reg=nreg,
                                elem_size=out_dim,
                                single_packet=False,
                                queue_num=gbuf_i % NQ,
                            )
                            for a in range(GRP):
                                tl = pp * GRP + a      # slab-local tile
                                t = sl * tps + tl      # dest tile
                                # sel[p, d, c] = (row[p, c] == d) * val[p, c]
                                # transposed layout: all last dims stride-1
                                # (DVE 2x mode); matmul reads sel[:, :, u].
                                sel = selpool.tile([128, 128, c_sub], BF16,
                                                   tag="sel")
                                row3 = row_sb[:, (tl * c_sub):(tl + 1) * c_sub] \
                                    .unsqueeze(1).to_broadcast([128, 128, c_sub])
                                val3 = val_sb[:, (tl * c_sub):(tl + 1) * c_sub] \
                                    .unsqueeze(1).to_broadcast([128, 128, c_sub])
                                nc.vector.tensor_tensor(sel[:], row3,
                                                        iota2_sb[:],
                                                        op=ALU.is_equal)
                                nc.vector.tensor_tensor(sel[:], sel[:], val3,
                                                        op=ALU.mult)
                                ps = p2psum.tile([128, out_dim], F32)
                                for u in range(c_sub):
                                    nc.tensor.matmul(
                                        ps[:], lhsT=sel[:, :, u],
                                        rhs=gt[:, a * c_sub + u, :],
                                        start=(u == 0), stop=(u == c_sub - 1))
                                if s == 0:
                                    # seed with the folded BN shift: saves an
                                    # add in the epilogue
                                    nc.vector.tensor_tensor(
                                        acc[:, t, :], ps[:], shift_sb[:],
                                        op=ALU.add)
                                elif s < 3:
                                    nc.vector.tensor_tensor(
                                        acc[:, t, :], acc[:, t, :], ps[:],
                                        op=ALU.add)
                                else:
                                    ob = selpool.tile([128, out_dim], F32,
                                                      tag="ob")
                                    nc.vector.tensor_tensor(ob[:], acc[:, t, :],
                                                            ps[:], op=ALU.add)
                                    ob3 = selpool.tile([128, out_dim], BF16,
                                                      tag="ob3")
                                    nc.scalar.activation(ob3[:], ob[:], AF.Gelu)
                                    nc.sync.dma_start(
                                        out[t * 128:(t + 1) * 128, :], ob3[:])

                # Pool queue: AG0 | pass0+1 gathers | AG1 | pass2+3 gathers.
                # Passes 0/1 read half-0 sections, passes 2/3 half-1, so only
                # passes that genuinely need AG1 queue behind it.
                p1_half(0)
                allgather(0)
                p1_half(1)
                idx_replicate(0)
                idx_replicate(1)
                idx_replicate(2)
                idx_replicate(3)
                if not skip_p2:
                    p2_pass(0)
                    p2_pass(1)
                # scheduling hint: don't let the greedy scheduler slot AG1
                # onto the Pool queue before pass-0/1's gathers (deps alone
                # would allow it at t~350us, serializing everything behind it)
                with tc.tile_wait_until(0.75):
                    allgather(1)
                if not skip_p2:
                    p2_pass(2)
                    p2_pass(3)

    nc.compile()
    return nc


def _preprocess(x, edge_row, edge_col, edge_val, W, b, gamma, beta,
                running_mean, running_var, bn_eps=1e-5):
    n, in_dim = x.shape
    out_dim = W.shape[1]
    assert n == N_CORES * SHARD
    nt = (SHARD + 127) // 128
    nt = ((nt + TPS - 1) // TPS) * TPS
    ng = 4 * nt

    inv_std = 1.0 / np.sqrt(running_var.astype(np.float64) + bn_eps)
    scale = (inv_std * gamma.astype(np.float64)).astype(np.float32)
    shift = (beta.astype(np.float64) - running_mean.astype(np.float64) * inv_std
             * gamma.astype(np.float64)).astype(np.float32)

    xb = x.astype(NPBF16)
    Wp = (W * scale[None, :]).astype(NPBF16)
    bp = np.ascontiguousarray(
        np.broadcast_to((b * scale).astype(np.float32), (128, out_dim)))
    shiftb = np.ascontiguousarray(np.broadcast_to(shift, (128, out_dim)))

    per_core = []
    c_sub = 1
    for m in range(N_CORES):
        lo, hi = m * SHARD, (m + 1) * SHARD
        mask = (edge_row >= lo) & (edge_row < hi)
        er = (edge_row[mask] - lo).astype(np.int64)
        ec = edge_col[mask].astype(np.int64)
        ev = edge_val[mask].astype(np.float32)
        src_core = ec // SHARD
        src_r = ec % SHARD
        h = src_r // HROWS
        sec = h * 2 + (src_core >> 2)
        loc = (src_core & 3) * HROWS + (src_r - h * HROWS)
        gid = sec * nt + (er >> 7)
        order = np.argsort(gid, kind="stable")
        er, ev, loc, gid = er[order], ev[order], loc[order], gid[order]
        counts = np.bincount(gid, minlength=ng)
        per_core.append((er, ev, loc, gid, counts))
        c_sub = max(c_sub, int(((counts + 127) // 128).max()))
    gsl = c_sub * 128
    nidx = GRP * gsl
    idxcols = nidx // 16
    npair = ng // GRP

    # iota2[p, d, c] = d  (transposed-selector compare plane)
    iota2 = np.ascontiguousarray(np.broadcast_to(
        np.repeat(np.arange(128, dtype=np.float32), c_sub).reshape(1, 128, c_sub),
        (128, 128, c_sub))).astype(NPBF16)

    in_maps = []
    for m in range(N_CORES):
        er, ev, loc, gid, counts = per_core[m]
        starts = np.zeros(ng, np.int64)
        np.cumsum(counts[:-1], out=starts[1:])
        rank = np.arange(len(er)) - starts[gid]
        rowp = np.zeros((128, ng * c_sub), NPBF16)
        valp = np.zeros((128, ng * c_sub), NPBF16)
        rowp[rank & 127, gid * c_sub + (rank >> 7)] = (er & 127).astype(NPBF16)
        valp[rank & 127, gid * c_sub + (rank >> 7)] = ev.astype(NPBF16)
        # merged-gather index stream: GRP tile-groups per gather; non-final
        # groups pad with VALID index 0 (rowp/valp stay 0 there, so the
        # selector zeroes those contributions); the final group pads with -1
        # which the count register trims.
        idx16 = np.full((16, npair * idxcols), -1, np.int16)
        idx16.reshape(16, npair, GRP, gsl // 16)[:, :, :GRP - 1, :] = 0
        slot = (gid % GRP) * gsl + rank
        idx16[slot & 15, (gid // GRP) * idxcols + (slot >> 4)] = \
            loc.astype(np.int16)
        cnts_arr = ((GRP - 1) * gsl +
                    counts.reshape(npair, GRP)[:, GRP - 1]).astype(np.int32)
        empty = np.nonzero(cnts_arr == 0)[0]
        if len(empty):
            idx16[0, empty * idxcols] = 0  # one dummy valid index, val stays 0
            cnts_arr[empty] = 1

        xTm = np.zeros((in_dim, SH), NPBF16)
        xTm[:, :SHARD] = xb[m * SHARD:(m + 1) * SHARD].T
        in_maps.append({
            "xT": np.ascontiguousarray(xTm),
            "Wp": Wp, "bp": bp, "shiftb": shiftb, "iota2": iota2,
            "idx16": np.ascontiguousarray(idx16),
            "rowp": np.ascontiguousarray(rowp),
            "valp": np.ascontiguousarray(valp),
            "cnts": cnts_arr.reshape(1, npair),
        })

    params = dict(in_dim=in_dim, out_dim=out_dim, nt=nt, c_sub=c_sub,
                  tps=TPS, xcols=XCOLS)
    return in_maps, params, SHARD


def kernel(x, edge_row, edge_col, edge_val, W, b, gamma, beta,
           running_mean, running_var):
    x = np.asarray(x)
    edge_row = np.asarray(edge_row)
    edge_col = np.asarray(edge_col)
    edge_val = np.asarray(edge_val)
    W = np.asarray(W)
    b = np.asarray(b)
    gamma = np.asarray(gamma)
    beta = np.asarray(beta)
    running_mean = np.asarray(running_mean)
    running_var = np.asarray(running_var)

    in_maps, params, shard = _preprocess(
        x, edge_row, edge_col, edge_val, W, b, gamma, beta,
        running_mean, running_var)
    nc = _build_program(**params)
    res = run_bass_kernel_spmd(nc, in_maps, core_ids=list(range(N_CORES)))
    outs = [res.results[m]["out"][:shard].astype(np.float32)
            for m in range(N_CORES)]
    return np.concatenate(outs, axis=0)
